# revision 88
# baseline (speedup 1.0000x reference)
"""GAT layer kernel for Trainium2, 8 NeuronCores, data-parallel.

Problem: nn_GATLayer (B=4, N=2048, F_IN=64, F_OUT=64, H=4).

Sharding: core c handles batch b = c//2 and destination-node rows
[ (c%2)*1024, (c%2)*1024+1024 ) of that batch (all heads, all source
nodes).  Every adjacency element is read exactly once across the 8
cores.  The host pre-transposes per-core tensors (mask, x) so the
device streams contiguous DMAs with no on-chip transposition of the
[N, N] mask.

Per-core algorithm (transposed-score layout, source node j on
partitions, destination node i on columns):
  h      = x @ W                        (PE, bf16)
  u_i    = h[i] . a_src[head],  v_j = h[j] . a_dst[head]
  Using exp(lrelu(s)) = max(exp(s), exp(0.2 s)) and the rank-1
  structure of s = u_i + v_j:
    e[j,i] = mask[j,i] * max(fu_i^5 * ev_j, fu_i * fv_j)
  with fu = exp(0.2 u), ev = exp(v), fv = exp(0.2 v) -- a single
  fused custom DVE pass per score tile, no Activation-engine exp over
  the NxN tensor.
  num/den: PSUM accumulation of  [h | 1]^T . e  over j-chunks  (PE)
  out    = num / den                    (transpose back, row scale)
"""

import sys

sys.path.insert(0, "/opt/trn_rl_repo")

from contextlib import ExitStack

import numpy as np
import ml_dtypes

import concourse.bass as bass
import concourse.mybir as mybir
import concourse.tile as tile
from concourse import bacc
from concourse.bass_utils import run_bass_kernel_spmd
from concourse.masks import make_identity

F32 = mybir.dt.float32
BF16 = mybir.dt.bfloat16
ACTF = mybir.ActivationFunctionType
NP_BF16 = ml_dtypes.bfloat16


# ---- custom DVE op: out = max(in0^5 * s0, in0 * s1) * in1 ----
def _register_gate5():
    import concourse.dve_ops as dve_ops
    from concourse.dve_ops import DveOp, _SUB_OPCODE_FOR_NAME, _CUSTOM_DVE_ROW_BASE
    from concourse.dve_spec import Spec, Src0, Src1, C0, C1, maxx, sq, lower
    from concourse.dve_uop import DveOpSpec

    name = "GATE5_MASK_MAX"
    if name in _SUB_OPCODE_FOR_NAME:
        return next(op for op in dve_ops.OPS if op.name == name)

    def _ref(in0, in1, s0, s1, imm2):
        f = in0.astype(np.float32)
        s0 = np.asarray(s0, np.float32).reshape(-1, 1)
        s1 = np.asarray(s1, np.float32).reshape(-1, 1)
        return np.maximum((f ** 5) * s0, f * s1) * in1.astype(np.float32)

    p5 = sq(sq(Src0)) * Src0
    spec = Spec(body=maxx(p5 * C0, Src0 * C1) * Src1, reference=_ref)
    row = _CUSTOM_DVE_ROW_BASE + len(dve_ops.OPS)
    assert row < 0x20
    _SUB_OPCODE_FOR_NAME[name] = row
    shas = {}
    for ver in ("v3", "v4"):
        uops = lower(spec, ver=ver)
        shas[ver] = DveOpSpec(name=name, opcode=row, uops=uops,
                              rd1_en=True).sha(ver)
    op = DveOp(name, spec, subdim=False, uops_sha=shas)
    dve_ops.OPS.append(op)
    dve_ops.CUSTOM_DVE_SPECS[name] = spec
    return op


GATE5 = _register_gate5()

B, N, F_IN, F_OUT, H = 4, 2048, 64, 64, 4
NI = N // 2            # destination rows per core
P = 128                # partitions
NJC = N // P           # 16 j-chunks
NIT = NI // P          # 8 i-tiles (per-core rows / 128)
HO = H * F_OUT         # 256
ALU = mybir.AluOpType
# j-chunks routed through the PE+ACT (mu -> prelu -> exp) pipeline instead
# of the fused DVE op, to offload the DVE bottleneck.  Head pair 1 gets
# more so its DVE stream (and thus the final epilogue) finishes earlier;
# ACT absorbs the extra work in its end-of-kernel idle window.
ALT_BY_HP = {0: (13, 14, 15), 1: (13, 14, 15)}
ALT_UNION = (13, 14, 15)
ALT_JCS = ALT_UNION  # any-ALT guard for the setup paths


def gat_core_program(tc, outs, ins):
    """Build the per-core program.  ins/outs are dicts of DRAM APs.

    ins:  mT  [N, NI] bf16  (transposed 0/1 mask: mT[j, i] = adj[i0+i, j])
          xT  [F_IN, N] bf16  (x[b]^T, full batch-b node features)
          xiT [F_IN, NI] bf16 (this core's destination columns of xT)
          wb  [F_IN, H*F_OUT] bf16
          attnb [H, 2*F_OUT] bf16
    outs: out [NI, H*F_OUT] f32
    """
    nc = tc.nc
    ctx = ExitStack()
    mT_d, xT_d, xiT_d, wb_d, attn_d = (
        ins["mT"], ins["xT"], ins["xiT"], ins["wb"], ins["attnb"])
    out_d = outs["out"]

    const = ctx.enter_context(tc.tile_pool(name="const", bufs=1))

    # ---------------- persistent tensors ----------------
    identf = const.tile([P, P], F32)
    make_identity(nc, identf[:])
    identb = const.tile([P, P], BF16)
    make_identity(nc, identb[:])
    i200 = const.tile([P, P], BF16)               # 200*I (ALT mask matmul)
    nc.gpsimd.tensor_scalar_mul(i200[:], identb[:], 200.0)
    m01m1 = const.tile([P, max(1, len(ALT_UNION)), NI], BF16)  # mask-1 (0/-1)

    m01 = const.tile([P, NJC, NI], BF16)          # 32KB/part: mask, j on parts
    fubc = const.tile([P, H, NI], BF16)           # exp(0.2 u_i) bcast over j
    haug = const.tile([P, NJC, H, F_OUT + 1], BF16)
    evsc = const.tile([P, NJC, H], F32)           # exp(v_j)
    fvsc = const.tile([P, NJC, H], F32)           # exp(0.2 v_j)
    vraw = const.tile([P, NJC, H], F32)           # v_j (ALT pipeline)
    outf = const.tile([P, NIT, HO], F32)          # final output staging

    xT_sb = const.tile([F_IN, N], BF16)
    xiT_sb = const.tile([F_IN, NI], BF16)
    wb_sb = const.tile([F_IN, HO], BF16)
    vT_sb = const.tile([2 * H, N], F32)
    fu_sb = const.tile([H, NI], BF16)
    u_sb = const.tile([H, NI], BF16)
    wa = const.tile([F_IN, 2 * H], BF16)

    # ---------------- input DMAs ----------------
    # DMA triggers cost ~630ns serialized on HWDGE, so batch aggressively.
    # SP queue: xiT, then mask in 3 groups; ACT queue: wb, aa, xT, sel.
    aa = const.tile([P, 2, 2 * H], BF16)
    sel = const.tile([H, H, P], BF16)
    mT_blk = mT_d.rearrange("(s p) c -> p s c", p=P)
    nc.sync.dma_start(xiT_sb[:], xiT_d[:])
    nc.scalar.dma_start(wb_sb[:], wb_d[:])
    nc.sync.dma_start(m01[:, 0:2, :], mT_blk[:, 0:2, :])
    # aa layout [128, 2, 8] built on host (attnb is pre-arranged)
    nc.scalar.dma_start(aa[:], attn_d.rearrange("p (g c) -> p g c", g=2))
    nc.sync.dma_start(xT_sb[:], xT_d[:])
    nc.scalar.dma_start(sel[:], ins["selc"].rearrange("h (g p) -> h g p", p=P))
    # ALT chunks early (the Pool mask-shift + PE mu matmuls consume them)
    nc.sync.dma_start(m01[:, 12:14, :], mT_blk[:, 12:14, :])
    nc.sync.dma_start(m01[:, 14:16, :], mT_blk[:, 14:16, :])
    for s0 in range(2, 12, 2):
        nc.sync.dma_start(m01[:, s0:s0 + 2, :], mT_blk[:, s0:s0 + 2, :])
    # mask-1 (0/-1 bf16) for the ALT additive-mask matmul
    for k, jc in enumerate(ALT_UNION):
        nc.gpsimd.tensor_scalar(m01m1[:, k, :], m01[:, jc, :], 1.0, -1.0,
                                op0=ALU.mult, op1=ALU.add)

    # mu/h-matmul PSUM pool outlives the setup pools (stack order: enter
    # before sctx so sctx can close first)
    mu_ps = ctx.enter_context(tc.tile_pool(name="mups", bufs=2, space="PSUM"))
    sctx = ExitStack()
    sps = sctx.enter_context(tc.tile_pool(name="sps", bufs=2, space="PSUM"))
    ssb = sctx.enter_context(tc.tile_pool(name="ssb", bufs=2))

    # ---------------- W^T, wa = W @ AA ----------------
    wT = ssb.tile([P, 2, F_IN], BF16)
    for half in range(2):
        pt = sps.tile([P, F_IN], BF16, tag="sb")
        nc.tensor.transpose(pt[:], wb_sb[:, half * P:(half + 1) * P],
                            identb[:F_IN, :F_IN])
        nc.scalar.copy(wT[:, half, :], pt[:])
    pwa = sps.tile([F_IN, 2 * H], F32, tag="s")
    for half in range(2):
        nc.tensor.matmul(pwa[:], wT[:, half, :], aa[:, half, :],
                         start=(half == 0), stop=(half == 1))
    nc.scalar.copy(wa[:], pwa[:])

    # ---------------- v scalars: exp(v), exp(0.2 v), per 4-jc group ------
    vtp = sps.tile([P, NJC, 2 * H], F32, name="vtp", tag="vtp")

    def v_group(ch):
        pv = sps.tile([2 * H, 512], F32, tag="s")
        nc.tensor.matmul(pv[:], wa[:], xT_sb[:, ch * 512:(ch + 1) * 512],
                         start=True, stop=True)
        nc.scalar.copy(vT_sb[:, ch * 512:(ch + 1) * 512], pv[:])
        for j4 in range(4):
            jc = ch * 4 + j4
            nc.tensor.transpose(vtp[:, jc, :], vT_sb[:, jc * P:(jc + 1) * P],
                                identf[:2 * H, :2 * H])
        nc.scalar.activation(evsc[:, ch * 4:(ch + 1) * 4, :],
                             vtp[:, ch * 4:(ch + 1) * 4, H:2 * H], ACTF.Exp)
        nc.scalar.activation(fvsc[:, ch * 4:(ch + 1) * 4, :],
                             vtp[:, ch * 4:(ch + 1) * 4, H:2 * H], ACTF.Exp,
                             scale=0.2)

    # first v group early: it gates the first custom-DVE chunk
    v_group(0)

    # ---------------- u scalars (gate the main loop too) ----------------
    # fu = exp(0.2 u); fubc[h] = broadcast over partitions (heads 0,1 now,
    # 2,3 after the v-side work).
    for ch in range(NI // 512):
        pu = sps.tile([H, 512], F32, tag="s")
        nc.tensor.matmul(pu[:], wa[:, 0:H],
                         xiT_sb[:, ch * 512:(ch + 1) * 512],
                         start=True, stop=True)
        nc.scalar.activation(fu_sb[:, ch * 512:(ch + 1) * 512], pu[:],
                             ACTF.Exp, scale=0.2)
        if ALT_JCS:
            nc.scalar.copy(u_sb[:, ch * 512:(ch + 1) * 512], pu[:])

    def build_bc(h, src, dst):
        for ch in range(NI // 512):
            pb = sps.tile([P, 512], F32, tag="s")
            nc.tensor.matmul(pb[:], sel[:, h, :],
                             src[:, ch * 512:(ch + 1) * 512],
                             start=True, stop=True)
            nc.scalar.copy(dst[:, h, ch * 512:(ch + 1) * 512], pb[:])

    def build_fubc(h):
        build_bc(h, fu_sb, fubc)

    build_fubc(0)
    build_fubc(1)

    for ch in range(1, N // 512):
        v_group(ch)

    # ---------------- h = x @ W -> haug stationaries ----------------
    # (early in the ACT queue: the main-loop matmuls need haug[jc] soon)
    nc.gpsimd.memset(haug[:, :, :, F_OUT], 1.0)
    for s in range(NJC):
        ph = mu_ps.tile([P, 512], F32, tag="mu")
        nc.tensor.matmul(ph[:, 0:HO], xT_sb[:, s * P:(s + 1) * P], wb_sb[:],
                         start=True, stop=True)
        nc.scalar.copy(
            haug[:, s, :, 0:F_OUT],
            ph[:, 0:HO].rearrange("p (h f) -> p h f", h=H))

    # late setup (first needed ~25us in: ALT exps and head pair 1)
    if ALT_JCS:
        for ch in range(4):
            nc.scalar.copy(vraw[:, ch * 4:(ch + 1) * 4, :],
                           vtp[:, ch * 4:(ch + 1) * 4, H:2 * H])
    build_fubc(2)
    build_fubc(3)

    sctx.close()

    # ---------------- main: fused masked-exp-score -> matmul ----------------
    cpool = ctx.enter_context(tc.tile_pool(name="cwork", bufs=8))
    altc_pool = ctx.enter_context(
        tc.tile_pool(name="altc", bufs=max(1, 2 * len(ALT_UNION))))
    alt_sc = ctx.enter_context(tc.tile_pool(name="altsc", bufs=2))
    po_pool = ctx.enter_context(tc.tile_pool(name="po", bufs=1, space="PSUM"))
    pt_pool = ctx.enter_context(tc.tile_pool(name="ptrans", bufs=2, space="PSUM"))
    ot_pool = ctx.enter_context(tc.tile_pool(name="otsb", bufs=2))
    rec_pool = ctx.enter_context(tc.tile_pool(name="rec", bufs=2))

    def pos_matmuls(pos, jc, src, start, stop):
        for i in range(2):
            h = (jc_hp[0] * 2) + i
            for mh in range(NI // 512):
                nc.tensor.matmul(
                    pos[i][:, mh * 512:(mh + 1) * 512],
                    haug[:, jc, h, :],
                    src[:, i * NI + mh * 512:i * NI + (mh + 1) * 512],
                    start=start, stop=stop)

    # ALT producers: PE builds mu = 200*(m01-1) + u in PSUM; ACT does
    # prelu(mu + v_j) then exp straight into the ca tile -- zero DVE
    # work on these chunks (exp(lrelu(s)) == max(exp(s), exp(0.2 s))).
    alt_tiles = {}

    def alt_produce(php):
        for jc in ALT_BY_HP[php]:
            k = ALT_UNION.index(jc)
            ca = altc_pool.tile([P, 2 * NI], BF16, tag="ca")
            for i in range(2):
                h = php * 2 + i
                for half in range(2):
                    sl = slice(half * 512, (half + 1) * 512)
                    mups = mu_ps.tile([P, 512], F32, tag="mu")
                    nc.tensor.matmul(mups[:], i200[:], m01m1[:, k, sl],
                                     start=True, stop=False)
                    nc.tensor.matmul(mups[:], sel[:, h, :], u_sb[:, sl],
                                     start=False, stop=True)
                    lt = alt_sc.tile([P, 512], F32, tag="lt")
                    nc.scalar.activation(lt[:], mups[:], ACTF.Prelu,
                                         bias=vraw[:, jc, h:h + 1], alpha=0.2)
                    nc.scalar.activation(
                        ca[:, i * NI + half * 512:i * NI + (half + 1) * 512],
                        lt[:], ACTF.Exp)
            alt_tiles[(php, jc)] = ca

    jc_hp = [0]
    for hp in range(H // 2):
        jc_hp[0] = hp
        alt = ALT_BY_HP[hp]
        pos = [po_pool.tile([F_OUT + 1, NI], F32, name=f"po{hp}_{i}", tag=f"po{i}")
               for i in range(2)]

        def dve_chunk(jc):
            c2 = cpool.tile([P, 2 * NI], BF16, tag="c")
            for i in range(2):
                h = hp * 2 + i
                nc.vector._custom_dve(
                    GATE5, out=c2[:, i * NI:(i + 1) * NI],
                    in0=fubc[:, h, :], in1=m01[:, jc, :],
                    s0=evsc[:, jc, h:h + 1], s1=fvsc[:, jc, h:h + 1])
            pos_matmuls(pos, jc, c2, jc == 0,
                        (not alt) and jc == NJC - 1)

        # first chunks on the DVE path so PE has work queued before the
        # ALT-mu matmuls (whose PSUM pool rotates with the h-matmuls)
        for jc in range(8):
            dve_chunk(jc)
        if hp == 0:
            alt_produce(0)
        for jc in range(8, NJC):
            if jc in alt:
                continue
            dve_chunk(jc)
        for n, jc in enumerate(alt):
            pos_matmuls(pos, jc, alt_tiles[(hp, jc)], False,
                        n == len(alt) - 1)

        # epilogue for this head pair: transpose back (4 tiles per PSUM
        # group so transposes overlap the scale-copies), normalize
        for i in range(2):
            h = hp * 2 + i
            ot = ot_pool.tile([F_OUT + 1, NI], F32)
            nc.scalar.copy(ot[:], pos[i][:])
            for g in range(NIT // 4):
                ptp = pt_pool.tile([P, 4, F_OUT + 1], F32)
                for q in range(4):
                    it = g * 4 + q
                    nc.tensor.transpose(ptp[:, q, :],
                                        ot[:, it * P:(it + 1) * P],
                                        identf[:F_OUT + 1, :F_OUT + 1])
                rec = rec_pool.tile([P, 4], F32)
                nc.vector.reciprocal(rec[:], ptp[:, :, F_OUT])
                for q in range(4):
                    it = g * 4 + q
                    nc.scalar.activation(
                        outf[:, it, h * F_OUT:(h + 1) * F_OUT],
                        ptp[:, q, 0:F_OUT], ACTF.Copy,
                        scale=rec[:, q:q + 1])
        if hp == 0:
            # hp1's ALT tiles made right after hp0's epilogue is queued:
            # ACT fills its pre-hp1 idle window, hp1's tail shrinks
            alt_produce(1)

    out_blk = out_d.rearrange("(s p) c -> p s c", p=P)
    for t0 in range(0, NIT, 2):
        nc.sync.dma_start(out_blk[:, t0:t0 + 2, :], outf[:, t0:t0 + 2, :])
    ctx.close()


N_CORES = 8
_CACHE = {}


def _build(repeats=1):
    key = ("nc", repeats)
    if key not in _CACHE:
        nc = bacc.Bacc("TRN2", target_bir_lowering=False, debug=False,
                       num_devices=N_CORES)
        ins = {
            "mT": nc.dram_tensor("mT", [N, NI], BF16, kind="ExternalInput").ap(),
            "xT": nc.dram_tensor("xT", [F_IN, N], BF16, kind="ExternalInput").ap(),
            "xiT": nc.dram_tensor("xiT", [F_IN, NI], BF16,
                                  kind="ExternalInput").ap(),
            "wb": nc.dram_tensor("wb", [F_IN, HO], BF16,
                                 kind="ExternalInput").ap(),
            "attnb": nc.dram_tensor("attnb", [P, 4 * H], BF16,
                                    kind="ExternalInput").ap(),
            "selc": nc.dram_tensor("selc", [H, H * P], BF16,
                                   kind="ExternalInput").ap(),
        }
        outs = {"out": nc.dram_tensor("out", [NI, HO], F32,
                                      kind="ExternalOutput").ap()}
        with tile.TileContext(nc) as tc:
            for _ in range(repeats):
                gat_core_program(tc, outs, ins)
        nc.compile()
        _CACHE[key] = nc
    return _CACHE[key]


def make_in_maps(node_features, adj_matrix, W, attention):
    node_features = np.ascontiguousarray(node_features, dtype=np.float32)
    adj_matrix = np.ascontiguousarray(adj_matrix, dtype=np.int32)
    wb = np.ascontiguousarray(W, dtype=np.float32).astype(NP_BF16)
    # aa layout [128, 2*8]: aab[h*64+f mod 128, half(h)*8 + h] = a_src[h, f],
    # [..., half*8 + 4 + h] = a_dst[h, f]
    att = np.asarray(attention, dtype=np.float32)
    attnb = np.zeros((P, 4 * H), dtype=NP_BF16)
    for h in range(H):
        half, poff = divmod(h * F_OUT, P)
        attnb[poff:poff + F_OUT, half * 2 * H + h] = att[h, 0:F_OUT]
        attnb[poff:poff + F_OUT, half * 2 * H + H + h] = att[h, F_OUT:2 * F_OUT]
    selc = np.zeros((H, H * P), dtype=NP_BF16)
    for h in range(H):
        selc[h, h * P:(h + 1) * P] = 1
    in_maps = []
    for c in range(N_CORES):
        b, ih = divmod(c, 2)
        i0 = ih * NI
        # mask: mT[j, i] = adj[b, i0+i, j] as bf16 0/1
        mT_u16 = np.where(adj_matrix[b, i0:i0 + NI] != 0,
                          np.uint16(0x3F80), np.uint16(0)).T
        mT = np.ascontiguousarray(mT_u16).view(NP_BF16)
        xT = np.ascontiguousarray(node_features[b].T).astype(NP_BF16)
        xiT = np.ascontiguousarray(xT[:, i0:i0 + NI])
        in_maps.append({
            "mT": mT,
            "xT": xT,
            "xiT": xiT,
            "wb": wb,
            "attnb": attnb,
            "selc": selc,
        })
    return in_maps


def assemble(results):
    out = np.empty((B, N, H * F_OUT), dtype=np.float32)
    for c in range(N_CORES):
        b, ih = divmod(c, 2)
        i0 = ih * NI
        out[b, i0:i0 + NI] = results[c]["out"]
    return out


def kernel(node_features, adj_matrix, W, attention):
    nc = _build()
    in_maps = make_in_maps(node_features, adj_matrix, W, attention)
    res = run_bass_kernel_spmd(nc, in_maps, core_ids=list(range(N_CORES)))
    return assemble(res.results)


# revision 92
# speedup vs baseline: 1.6393x; 1.6393x over previous
"""GAT layer kernel for Trainium2, 8 NeuronCores, data-parallel.

Problem: nn_GATLayer (B=4, N=2048, F_IN=64, F_OUT=64, H=4).

Sharding: core c handles batch b = c//2 and destination-node rows
[ (c%2)*1024, (c%2)*1024+1024 ) of that batch (all heads, all source
nodes).  Every adjacency element is read exactly once across the 8
cores.  The host pre-transposes per-core tensors (mask, x) so the
device streams contiguous DMAs with no on-chip transposition of the
[N, N] mask.

Per-core algorithm (transposed-score layout, source node j on
partitions, destination node i on columns):
  h      = x @ W                        (PE, bf16)
  u_i    = h[i] . a_src[head],  v_j = h[j] . a_dst[head]
  Using exp(lrelu(s)) = max(exp(s), exp(0.2 s)) and the rank-1
  structure of s = u_i + v_j:
    e[j,i] = mask[j,i] * max(fu_i^5 * ev_j, fu_i * fv_j)
  with fu = exp(0.2 u), ev = exp(v), fv = exp(0.2 v) -- a single
  fused custom DVE pass per score tile, no Activation-engine exp over
  the NxN tensor.
  num/den: PSUM accumulation of  [h | 1]^T . e  over j-chunks  (PE)
  out    = num / den                    (transpose back, row scale)
"""

import sys

sys.path.insert(0, "/opt/trn_rl_repo")

from contextlib import ExitStack

import numpy as np
import ml_dtypes

import concourse.bass as bass
import concourse.mybir as mybir
import concourse.tile as tile
from concourse import bacc
from concourse.bass_utils import run_bass_kernel_spmd
from concourse.masks import make_identity

F32 = mybir.dt.float32
BF16 = mybir.dt.bfloat16
ACTF = mybir.ActivationFunctionType
NP_BF16 = ml_dtypes.bfloat16


# ---- custom DVE op: out = max(in0^5 * s0, in0 * s1) * in1 ----
def _register_gate5():
    import concourse.dve_ops as dve_ops
    from concourse.dve_ops import DveOp, _SUB_OPCODE_FOR_NAME, _CUSTOM_DVE_ROW_BASE
    from concourse.dve_spec import Spec, Src0, Src1, C0, C1, maxx, sq, lower
    from concourse.dve_uop import DveOpSpec

    name = "GATE5_MASK_MAX"
    if name in _SUB_OPCODE_FOR_NAME:
        return next(op for op in dve_ops.OPS if op.name == name)

    def _ref(in0, in1, s0, s1, imm2):
        f = in0.astype(np.float32)
        s0 = np.asarray(s0, np.float32).reshape(-1, 1)
        s1 = np.asarray(s1, np.float32).reshape(-1, 1)
        return np.maximum((f ** 5) * s0, f * s1) * in1.astype(np.float32)

    p5 = sq(sq(Src0)) * Src0
    spec = Spec(body=maxx(p5 * C0, Src0 * C1) * Src1, reference=_ref)
    row = _CUSTOM_DVE_ROW_BASE + len(dve_ops.OPS)
    assert row < 0x20
    _SUB_OPCODE_FOR_NAME[name] = row
    shas = {}
    for ver in ("v3", "v4"):
        uops = lower(spec, ver=ver)
        shas[ver] = DveOpSpec(name=name, opcode=row, uops=uops,
                              rd1_en=True).sha(ver)
    op = DveOp(name, spec, subdim=False, uops_sha=shas)
    dve_ops.OPS.append(op)
    dve_ops.CUSTOM_DVE_SPECS[name] = spec
    return op


GATE5 = _register_gate5()

B, N, F_IN, F_OUT, H = 4, 2048, 64, 64, 4
NI = N // 2            # destination rows per core
P = 128                # partitions
NJC = N // P           # 16 j-chunks
NIT = NI // P          # 8 i-tiles (per-core rows / 128)
HO = H * F_OUT         # 256
ALU = mybir.AluOpType
# j-chunks routed through the PE+ACT (mu -> prelu -> exp) pipeline instead
# of the fused DVE op, to offload the DVE bottleneck.  Head pair 1 gets
# more so its DVE stream (and thus the final epilogue) finishes earlier;
# ACT absorbs the extra work in its end-of-kernel idle window.
ALT_BY_HP = {0: (13, 14, 15), 1: (13, 14, 15)}
ALT_UNION = (13, 14, 15)
ALT_JCS = ALT_UNION  # any-ALT guard for the setup paths


def gat_core_program(tc, outs, ins):
    """Build the per-core program.  ins/outs are dicts of DRAM APs.

    ins:  mT  [N, NI] bf16  (transposed 0/1 mask: mT[j, i] = adj[i0+i, j])
          xT  [F_IN, N] bf16  (x[b]^T, full batch-b node features)
          xiT [F_IN, NI] bf16 (this core's destination columns of xT)
          wb  [F_IN, H*F_OUT] bf16
          attnb [H, 2*F_OUT] bf16
    outs: out [NI, H*F_OUT] f32
    """
    nc = tc.nc
    ctx = ExitStack()
    mT_d, xT_d, xiT_d, wb_d, attn_d = (
        ins["mT"], ins["xT"], ins["xiT"], ins["wb"], ins["attnb"])
    out_d = outs["out"]

    const = ctx.enter_context(tc.tile_pool(name="const", bufs=1))

    # ---------------- persistent tensors ----------------
    identf = const.tile([P, P], F32)
    make_identity(nc, identf[:])
    identb = const.tile([P, P], BF16)
    make_identity(nc, identb[:])
    i200 = const.tile([P, P], BF16)               # 200*I (ALT mask matmul)
    nc.gpsimd.tensor_scalar_mul(i200[:], identb[:], 200.0)
    m01m1 = const.tile([P, max(1, len(ALT_UNION)), NI], BF16)  # mask-1 (0/-1)

    m01 = const.tile([P, NJC, NI], BF16)          # 32KB/part: mask, j on parts
    fubc = const.tile([P, H, NI], BF16)           # exp(0.2 u_i) bcast over j
    haug = const.tile([P, NJC, H, F_OUT + 1], BF16)
    evsc = const.tile([P, NJC, H], F32)           # exp(v_j)
    fvsc = const.tile([P, NJC, H], F32)           # exp(0.2 v_j)
    vraw = const.tile([P, NJC, H], F32)           # v_j (ALT pipeline)
    outf = const.tile([P, NIT, HO], F32)          # final output staging

    xT_sb = const.tile([F_IN, N], BF16)
    xiT_sb = const.tile([F_IN, NI], BF16)
    wb_sb = const.tile([F_IN, HO], BF16)
    vT_sb = const.tile([2 * H, N], F32)
    fu_sb = const.tile([H, NI], BF16)
    u_sb = const.tile([H, NI], BF16)
    wa = const.tile([F_IN, 2 * H], BF16)

    # ---------------- input DMAs ----------------
    # DMA triggers cost ~630ns serialized on HWDGE, so batch aggressively.
    # SP queue: xiT, then mask in 3 groups; ACT queue: wb, aa, xT, sel.
    aa = const.tile([P, 2, 2 * H], BF16)
    sel = const.tile([H, H, P], BF16)
    mT_blk = mT_d.rearrange("(s p) c -> p s c", p=P)
    nc.sync.dma_start(xiT_sb[:], xiT_d[:])
    nc.scalar.dma_start(wb_sb[:], wb_d[:])
    nc.sync.dma_start(m01[:, 0:2, :], mT_blk[:, 0:2, :])
    # aa layout [128, 2, 8] built on host (attnb is pre-arranged)
    nc.scalar.dma_start(aa[:], attn_d.rearrange("p (g c) -> p g c", g=2))
    nc.sync.dma_start(xT_sb[:], xT_d[:])
    nc.scalar.dma_start(sel[:], ins["selc"].rearrange("h (g p) -> h g p", p=P))
    # ALT chunks early (the Pool mask-shift + PE mu matmuls consume them)
    nc.sync.dma_start(m01[:, 12:14, :], mT_blk[:, 12:14, :])
    nc.sync.dma_start(m01[:, 14:16, :], mT_blk[:, 14:16, :])
    for s0 in range(2, 12, 2):
        nc.sync.dma_start(m01[:, s0:s0 + 2, :], mT_blk[:, s0:s0 + 2, :])
    # mask-1 (0/-1 bf16) for the ALT additive-mask matmul
    for k, jc in enumerate(ALT_UNION):
        nc.gpsimd.tensor_scalar(m01m1[:, k, :], m01[:, jc, :], 1.0, -1.0,
                                op0=ALU.mult, op1=ALU.add)

    # mu/h-matmul PSUM pool outlives the setup pools (stack order: enter
    # before sctx so sctx can close first)
    mu_ps = ctx.enter_context(tc.tile_pool(name="mups", bufs=2, space="PSUM"))
    sctx = ExitStack()
    sps = sctx.enter_context(tc.tile_pool(name="sps", bufs=2, space="PSUM"))
    ssb = sctx.enter_context(tc.tile_pool(name="ssb", bufs=2))

    # ---------------- W^T, wa = W @ AA ----------------
    wT = ssb.tile([P, 2, F_IN], BF16)
    for half in range(2):
        pt = sps.tile([P, F_IN], BF16, tag="sb")
        nc.tensor.transpose(pt[:], wb_sb[:, half * P:(half + 1) * P],
                            identb[:F_IN, :F_IN])
        nc.scalar.copy(wT[:, half, :], pt[:])
    pwa = sps.tile([F_IN, 2 * H], F32, tag="s")
    for half in range(2):
        nc.tensor.matmul(pwa[:], wT[:, half, :], aa[:, half, :],
                         start=(half == 0), stop=(half == 1))
    nc.scalar.copy(wa[:], pwa[:])

    # ---------------- v scalars: exp(v), exp(0.2 v), per 4-jc group ------
    vtp = sps.tile([P, NJC, 2 * H], F32, name="vtp", tag="vtp")

    def v_group(ch):
        pv = sps.tile([2 * H, 512], F32, tag="s")
        nc.tensor.matmul(pv[:], wa[:], xT_sb[:, ch * 512:(ch + 1) * 512],
                         start=True, stop=True)
        nc.scalar.copy(vT_sb[:, ch * 512:(ch + 1) * 512], pv[:])
        for j4 in range(4):
            jc = ch * 4 + j4
            nc.tensor.transpose(vtp[:, jc, :], vT_sb[:, jc * P:(jc + 1) * P],
                                identf[:2 * H, :2 * H])
        nc.scalar.activation(evsc[:, ch * 4:(ch + 1) * 4, :],
                             vtp[:, ch * 4:(ch + 1) * 4, H:2 * H], ACTF.Exp)
        nc.scalar.activation(fvsc[:, ch * 4:(ch + 1) * 4, :],
                             vtp[:, ch * 4:(ch + 1) * 4, H:2 * H], ACTF.Exp,
                             scale=0.2)

    # first v group early: it gates the first custom-DVE chunk
    v_group(0)

    # ---------------- u scalars (gate the main loop too) ----------------
    # fu = exp(0.2 u); fubc[h] = broadcast over partitions (heads 0,1 now,
    # 2,3 after the v-side work).
    for ch in range(NI // 512):
        pu = sps.tile([H, 512], F32, tag="s")
        nc.tensor.matmul(pu[:], wa[:, 0:H],
                         xiT_sb[:, ch * 512:(ch + 1) * 512],
                         start=True, stop=True)
        nc.scalar.activation(fu_sb[:, ch * 512:(ch + 1) * 512], pu[:],
                             ACTF.Exp, scale=0.2)
        if ALT_JCS:
            nc.scalar.copy(u_sb[:, ch * 512:(ch + 1) * 512], pu[:])

    def build_bc(h, src, dst):
        for ch in range(NI // 512):
            pb = sps.tile([P, 512], F32, tag="s")
            nc.tensor.matmul(pb[:], sel[:, h, :],
                             src[:, ch * 512:(ch + 1) * 512],
                             start=True, stop=True)
            nc.scalar.copy(dst[:, h, ch * 512:(ch + 1) * 512], pb[:])

    def build_fubc(h):
        build_bc(h, fu_sb, fubc)

    build_fubc(0)
    build_fubc(1)

    for ch in range(1, N // 512):
        v_group(ch)

    # ---------------- h = x @ W -> haug stationaries ----------------
    # (early in the ACT queue: the main-loop matmuls need haug[jc] soon)
    nc.gpsimd.memset(haug[:, :, :, F_OUT], 1.0)
    for s in range(NJC):
        ph = mu_ps.tile([P, 512], F32, tag="mu")
        nc.tensor.matmul(ph[:, 0:HO], xT_sb[:, s * P:(s + 1) * P], wb_sb[:],
                         start=True, stop=True)
        nc.scalar.copy(
            haug[:, s, :, 0:F_OUT],
            ph[:, 0:HO].rearrange("p (h f) -> p h f", h=H))

    # late setup (first needed ~25us in: ALT exps and head pair 1)
    if ALT_JCS:
        for ch in range(4):
            nc.scalar.copy(vraw[:, ch * 4:(ch + 1) * 4, :],
                           vtp[:, ch * 4:(ch + 1) * 4, H:2 * H])
    build_fubc(2)
    build_fubc(3)

    sctx.close()

    # ---------------- main: fused masked-exp-score -> matmul ----------------
    cpool = ctx.enter_context(tc.tile_pool(name="cwork", bufs=8))
    altc_pool = ctx.enter_context(
        tc.tile_pool(name="altc", bufs=max(1, 2 * len(ALT_UNION))))
    alt_sc = ctx.enter_context(tc.tile_pool(name="altsc", bufs=4))
    po_pool = ctx.enter_context(tc.tile_pool(name="po", bufs=1, space="PSUM"))
    pt_pool = ctx.enter_context(tc.tile_pool(name="ptrans", bufs=2, space="PSUM"))
    ot_pool = ctx.enter_context(tc.tile_pool(name="otsb", bufs=2))
    rec_pool = ctx.enter_context(tc.tile_pool(name="rec", bufs=2))

    def pos_matmuls(pos, jc, src, start, stop):
        for i in range(2):
            h = (jc_hp[0] * 2) + i
            for mh in range(NI // 512):
                nc.tensor.matmul(
                    pos[i][:, mh * 512:(mh + 1) * 512],
                    haug[:, jc, h, :],
                    src[:, i * NI + mh * 512:i * NI + (mh + 1) * 512],
                    start=start, stop=stop)

    # ALT producers: PE builds mu = 200*(m01-1) + u in PSUM; ACT does
    # prelu(mu + v_j) then exp straight into the ca tile -- zero DVE
    # work on these chunks (exp(lrelu(s)) == max(exp(s), exp(0.2 s))).
    alt_tiles = {}

    def alt_produce(php):
        for jc in ALT_BY_HP[php]:
            k = ALT_UNION.index(jc)
            ca = altc_pool.tile([P, 2 * NI], BF16, tag="ca")
            for i in range(2):
                h = php * 2 + i
                for half in range(2):
                    sl = slice(half * 512, (half + 1) * 512)
                    mups = mu_ps.tile([P, 512], F32, tag="mu")
                    nc.tensor.matmul(mups[:], i200[:], m01m1[:, k, sl],
                                     start=True, stop=False)
                    nc.tensor.matmul(mups[:], sel[:, h, :], u_sb[:, sl],
                                     start=False, stop=True)
                    lt = alt_sc.tile([P, 512], F32, tag="lt")
                    nc.scalar.activation(lt[:], mups[:], ACTF.Prelu,
                                         bias=vraw[:, jc, h:h + 1], alpha=0.2)
                    nc.scalar.activation(
                        ca[:, i * NI + half * 512:i * NI + (half + 1) * 512],
                        lt[:], ACTF.Exp)
            alt_tiles[(php, jc)] = ca

    jc_hp = [0]
    for hp in range(H // 2):
        jc_hp[0] = hp
        alt = ALT_BY_HP[hp]
        pos = [po_pool.tile([F_OUT + 1, NI], F32, name=f"po{hp}_{i}", tag=f"po{i}")
               for i in range(2)]

        def dve_chunk(jc):
            c2 = cpool.tile([P, 2 * NI], BF16, tag="c")
            for i in range(2):
                h = hp * 2 + i
                nc.vector._custom_dve(
                    GATE5, out=c2[:, i * NI:(i + 1) * NI],
                    in0=fubc[:, h, :], in1=m01[:, jc, :],
                    s0=evsc[:, jc, h:h + 1], s1=fvsc[:, jc, h:h + 1])
            pos_matmuls(pos, jc, c2, jc == 0,
                        (not alt) and jc == NJC - 1)

        # first chunks on the DVE path so PE has work queued before the
        # ALT-mu matmuls (whose PSUM pool rotates with the h-matmuls)
        for jc in range(6):
            dve_chunk(jc)
        if hp == 0:
            alt_produce(0)
        for jc in range(6, NJC):
            if jc in alt:
                continue
            dve_chunk(jc)
        for n, jc in enumerate(alt):
            pos_matmuls(pos, jc, alt_tiles[(hp, jc)], False,
                        n == len(alt) - 1)

        # epilogue for this head pair: transpose back (4 tiles per PSUM
        # group so transposes overlap the scale-copies), normalize
        for i in range(2):
            h = hp * 2 + i
            ot = ot_pool.tile([F_OUT + 1, NI], F32)
            nc.scalar.copy(ot[:], pos[i][:])
            for g in range(NIT // 4):
                ptp = pt_pool.tile([P, 4, F_OUT + 1], F32)
                for q in range(4):
                    it = g * 4 + q
                    nc.tensor.transpose(ptp[:, q, :],
                                        ot[:, it * P:(it + 1) * P],
                                        identf[:F_OUT + 1, :F_OUT + 1])
                rec = rec_pool.tile([P, 4], F32)
                nc.vector.reciprocal(rec[:], ptp[:, :, F_OUT])
                for q in range(4):
                    it = g * 4 + q
                    nc.scalar.activation(
                        outf[:, it, h * F_OUT:(h + 1) * F_OUT],
                        ptp[:, q, 0:F_OUT], ACTF.Copy,
                        scale=rec[:, q:q + 1])
        if hp == 0:
            # hp1's ALT tiles made right after hp0's epilogue is queued:
            # ACT fills its pre-hp1 idle window, hp1's tail shrinks
            alt_produce(1)

    out_blk = out_d.rearrange("(s p) c -> p s c", p=P)
    for t0 in range(0, NIT, 2):
        nc.sync.dma_start(out_blk[:, t0:t0 + 2, :], outf[:, t0:t0 + 2, :])
    ctx.close()


N_CORES = 8
_CACHE = {}


def _build(repeats=1):
    key = ("nc", repeats)
    if key not in _CACHE:
        nc = bacc.Bacc("TRN2", target_bir_lowering=False, debug=False,
                       num_devices=N_CORES)
        ins = {
            "mT": nc.dram_tensor("mT", [N, NI], BF16, kind="ExternalInput").ap(),
            "xT": nc.dram_tensor("xT", [F_IN, N], BF16, kind="ExternalInput").ap(),
            "xiT": nc.dram_tensor("xiT", [F_IN, NI], BF16,
                                  kind="ExternalInput").ap(),
            "wb": nc.dram_tensor("wb", [F_IN, HO], BF16,
                                 kind="ExternalInput").ap(),
            "attnb": nc.dram_tensor("attnb", [P, 4 * H], BF16,
                                    kind="ExternalInput").ap(),
            "selc": nc.dram_tensor("selc", [H, H * P], BF16,
                                   kind="ExternalInput").ap(),
        }
        outs = {"out": nc.dram_tensor("out", [NI, HO], F32,
                                      kind="ExternalOutput").ap()}
        with tile.TileContext(nc) as tc:
            for _ in range(repeats):
                gat_core_program(tc, outs, ins)
        nc.compile()
        _CACHE[key] = nc
    return _CACHE[key]


def make_in_maps(node_features, adj_matrix, W, attention):
    node_features = np.ascontiguousarray(node_features, dtype=np.float32)
    adj_matrix = np.ascontiguousarray(adj_matrix, dtype=np.int32)
    wb = np.ascontiguousarray(W, dtype=np.float32).astype(NP_BF16)
    # aa layout [128, 2*8]: aab[h*64+f mod 128, half(h)*8 + h] = a_src[h, f],
    # [..., half*8 + 4 + h] = a_dst[h, f]
    att = np.asarray(attention, dtype=np.float32)
    attnb = np.zeros((P, 4 * H), dtype=NP_BF16)
    for h in range(H):
        half, poff = divmod(h * F_OUT, P)
        attnb[poff:poff + F_OUT, half * 2 * H + h] = att[h, 0:F_OUT]
        attnb[poff:poff + F_OUT, half * 2 * H + H + h] = att[h, F_OUT:2 * F_OUT]
    selc = np.zeros((H, H * P), dtype=NP_BF16)
    for h in range(H):
        selc[h, h * P:(h + 1) * P] = 1
    in_maps = []
    for c in range(N_CORES):
        b, ih = divmod(c, 2)
        i0 = ih * NI
        # mask: mT[j, i] = adj[b, i0+i, j] as bf16 0/1
        mT_u16 = np.where(adj_matrix[b, i0:i0 + NI] != 0,
                          np.uint16(0x3F80), np.uint16(0)).T
        mT = np.ascontiguousarray(mT_u16).view(NP_BF16)
        xT = np.ascontiguousarray(node_features[b].T).astype(NP_BF16)
        xiT = np.ascontiguousarray(xT[:, i0:i0 + NI])
        in_maps.append({
            "mT": mT,
            "xT": xT,
            "xiT": xiT,
            "wb": wb,
            "attnb": attnb,
            "selc": selc,
        })
    return in_maps


def assemble(results):
    out = np.empty((B, N, H * F_OUT), dtype=np.float32)
    for c in range(N_CORES):
        b, ih = divmod(c, 2)
        i0 = ih * NI
        out[b, i0:i0 + NI] = results[c]["out"]
    return out


def kernel(node_features, adj_matrix, W, attention):
    nc = _build()
    in_maps = make_in_maps(node_features, adj_matrix, W, attention)
    res = run_bass_kernel_spmd(nc, in_maps, core_ids=list(range(N_CORES)))
    return assemble(res.results)
